# revision 4
# baseline (speedup 1.0000x reference)
"""EngagementBiasedMHA on 8 Trainium2 NeuronCores.

Sharding: 4 batches x 2 head-groups (8 heads each).  Each core computes, for
its (batch, head-group):
  - Q^T/K^T projections in [feat, token] layout (feature dim on partitions)
  - V projection in [token, feat] layout (+ a ones column for the softmax
    denominator)
  - attention in transposed layout: S^T = K @ Q^T with keys on partitions, so
    the per-key engagement bias/mask folds into the Exp activation as a
    per-partition bias, and exp(S^T) is already the correct (lhs-contraction)
    layout for the PV matmul
  - O^T = Vhat^T @ P^T accumulated over key tiles; the ones column of Vhat
    yields softmax denominators as row 64 of the PSUM result
  - row-parallel partial output projection y_partial = O_hg @ out_w.T[hg]
Host side: transpose/slice inputs per core, then sum the two partial outputs
per batch (row-parallel unshard).
"""

import sys

if "/opt/trn_rl_repo" not in sys.path:
    sys.path.insert(0, "/opt/trn_rl_repo")

import numpy as np
from concourse import bacc, tile
import concourse.mybir as mybir
from concourse.bass_utils import run_bass_kernel_spmd

F32 = mybir.dt.float32
AF = mybir.ActivationFunctionType

B, T, D, H = 4, 2048, 1024, 16
HD = 64
HG = 8           # heads per core
NKT = T // 128   # 16 key/token tiles
NQC = T // 512   # 4 query chunks
NDT = D // 128   # 8 d_in tiles
VROW = HG * (HD + 1)  # 520 Vhat columns per key tile

_cache = {}

# Results of the most recent run (for the test harness to read exec times).
last_results = None


def _build_program():
    nc = bacc.Bacc("TRN2", target_bir_lowering=False, debug=False, num_devices=8)
    xt_d = nc.declare_dram_parameter("xt", [D, T], F32, isOutput=False)
    # wqk: row block m*128+p holds, at col d*128+f, weight qkv_w.T[d*128+p, feat(m)+f]
    wqk_d = nc.declare_dram_parameter("wqk", [1024, 1024], F32, isOutput=False)
    wv_d = nc.declare_dram_parameter("wv", [D, 512], F32, isOutput=False)
    bqk_d = nc.declare_dram_parameter("bqk", [128, 8], F32, isOutput=False)
    bv_d = nc.declare_dram_parameter("bv", [128, 512], F32, isOutput=False)
    eng_d = nc.declare_dram_parameter("eng", [128, NKT], F32, isOutput=False)
    maskf_d = nc.declare_dram_parameter("maskf", [128, NKT], F32, isOutput=False)
    wo_d = nc.declare_dram_parameter("wo", [512, 1024], F32, isOutput=False)
    bo_d = nc.declare_dram_parameter("bo", [128, 1024], F32, isOutput=False)
    y_d = nc.declare_dram_parameter("y", [T, D], F32, isOutput=True)

    with tile.TileContext(nc) as tc:
        with (
            tc.tile_pool(name="persist", bufs=1) as persist,
            tc.tile_pool(name="xtpool", bufs=2) as xtpool,
            tc.tile_pool(name="wqkpool", bufs=2) as wqkpool,
            tc.tile_pool(name="wpool", bufs=1) as wpool,
            tc.tile_pool(name="small", bufs=1) as small,
            tc.tile_pool(name="ptpool", bufs=4) as ptpool,
            tc.tile_pool(name="otpool", bufs=6) as otpool,
            tc.tile_pool(name="evacpool", bufs=3) as evacpool,
            tc.tile_pool(name="recpool", bufs=2) as recpool,
            tc.tile_pool(name="psA", bufs=2, space="PSUM") as psA,
            tc.tile_pool(name="psST", bufs=3, space="PSUM") as psST,
            tc.tile_pool(name="psOT", bufs=3, space="PSUM") as psOT,
        ):
            # ---- small inputs ----
            BQK = small.tile([128, 8], F32, name="BQK")
            nc.sync.dma_start(BQK[:], bqk_d[:])
            BV = small.tile([128, 512], F32, name="BV")
            nc.sync.dma_start(BV[:], bv_d[:])
            ENG = small.tile([128, NKT], F32, name="ENG")
            nc.sync.dma_start(ENG[:], eng_d[:])
            MSK = small.tile([128, NKT], F32, name="MSK")
            nc.sync.dma_start(MSK[:], maskf_d[:])
            BO = small.tile([128, 1024], F32, name="BO")
            nc.sync.dma_start(BO[:], bo_d[:])

            # ---- per-key bias: BK = ln(max(eng, 1e-6)) - 1e9 * mask ----
            BK = small.tile([128, NKT], F32, name="BK")
            nc.vector.tensor_scalar_max(BK[:], ENG[:], 1e-6)
            nc.scalar.activation(BK[:], BK[:], AF.Ln)
            MK9 = small.tile([128, NKT], F32, name="MK9")
            nc.vector.tensor_scalar_mul(MK9[:], MSK[:], -1e9)
            nc.vector.tensor_add(BK[:], BK[:], MK9[:])

            # ---- phase 1: projections, chunked over 512-token chunks ----
            QTKT = persist.tile([128, 8 * T], F32, name="QTKT")
            VHAT = persist.tile([128, NKT * VROW], F32, name="VHAT")
            nc.vector.memset(VHAT[:], 1.0)
            WV = wpool.tile([128, NDT * 512], F32, name="WV", tag="wv_wo")
            for d in range(NDT):
                nc.sync.dma_start(WV[:, d * 512:(d + 1) * 512], wv_d[d * 128:(d + 1) * 128, :])

            for c in range(NQC):
                XTc = xtpool.tile([128, NDT * 512], F32, name="XTc", tag="xtc")
                for d in range(NDT):
                    nc.sync.dma_start(XTc[:, d * 512:(d + 1) * 512],
                                      xt_d[d * 128:(d + 1) * 128, c * 512:(c + 1) * 512])
                # Q^T / K^T features (8 tiles of 128 feats each)
                for m in range(8):
                    WQKm = wqkpool.tile([128, 1024], F32, name="WQKm", tag="wqk")
                    nc.sync.dma_start(WQKm[:], wqk_d[m * 128:(m + 1) * 128, :])
                    ps = psA.tile([128, 512], F32, name="ps_qk", tag="a")
                    for d in range(NDT):
                        nc.tensor.matmul(
                            ps[:],
                            lhsT=WQKm[:, d * 128:(d + 1) * 128],
                            rhs=XTc[:, d * 512:(d + 1) * 512],
                            start=(d == 0), stop=(d == NDT - 1),
                        )
                    nc.vector.tensor_scalar_add(
                        QTKT[:, m * T + c * 512: m * T + c * 512 + 512],
                        ps[:], BQK[:, m:m + 1])
                # V for this chunk's 4 token tiles
                for t4 in range(4):
                    t = c * 4 + t4
                    ps = psA.tile([128, 512], F32, name="ps_v", tag="a")
                    for d in range(NDT):
                        nc.tensor.matmul(
                            ps[:],
                            lhsT=XTc[:, d * 512 + t4 * 128: d * 512 + (t4 + 1) * 128],
                            rhs=WV[:, d * 512:(d + 1) * 512],
                            start=(d == 0), stop=(d == NDT - 1),
                        )
                    for h in range(HG):
                        nc.vector.tensor_add(
                            VHAT[:, t * VROW + h * 65: t * VROW + h * 65 + 64],
                            ps[:, h * 64:(h + 1) * 64], BV[:, h * 64:(h + 1) * 64])

            WO = wpool.tile([128, 4 * 1024], F32, name="WO", tag="wv_wo")
            for f in range(4):
                nc.sync.dma_start(WO[:, f * 1024:(f + 1) * 1024], wo_d[f * 128:(f + 1) * 128, :])

            # ---- phase 2+3: attention (transposed layout) + output projection ----
            for qc in range(NQC):
                otc = []
                for hp in range(4):
                    qt = hp
                    ktf = 4 + hp
                    op0 = psOT.tile([65, 512], F32, name="op0", tag="ot")
                    op1 = psOT.tile([65, 512], F32, name="op1", tag="ot")
                    ops = (op0, op1)
                    for kt in range(NKT):
                        for sub in range(2):
                            h = 2 * hp + sub
                            lo = sub * 64
                            st = psST.tile([128, 512], F32, name="st", tag="st")
                            nc.tensor.matmul(
                                st[:],
                                lhsT=QTKT[lo:lo + 64, ktf * T + kt * 128: ktf * T + (kt + 1) * 128],
                                rhs=QTKT[lo:lo + 64, qt * T + qc * 512: qt * T + qc * 512 + 512],
                                start=True, stop=True)
                            pt = ptpool.tile([128, 512], F32, name="pt", tag="pt")
                            nc.scalar.activation(
                                pt[:], st[:], AF.Exp,
                                bias=BK[:, kt:kt + 1], scale=0.125)
                            nc.tensor.matmul(
                                ops[sub][:],
                                lhsT=VHAT[:, kt * VROW + h * 65: kt * VROW + (h + 1) * 65],
                                rhs=pt[:],
                                start=(kt == 0), stop=(kt == NKT - 1))
                    OTc = otpool.tile([128, 512], F32, name="OTc", tag="otc")
                    for sub in range(2):
                        rec = recpool.tile([1, 512], F32, name="rec", tag="rec")
                        nc.vector.reciprocal(rec[:], ops[sub][64:65, :])
                        bcast = recpool.tile([64, 512], F32, name="bcast", tag="bcast")
                        nc.gpsimd.partition_broadcast(bcast[:], rec[0:1, :])
                        nc.vector.tensor_mul(
                            OTc[sub * 64:sub * 64 + 64, :],
                            ops[sub][0:64, :], bcast[:])
                    otc.append(OTc)
                # output projection for this 512-token chunk
                for t4 in range(4):
                    tt = qc * 4 + t4
                    for c2 in range(2):
                        ps = psST.tile([128, 512], F32, name="ps_y", tag="st")
                        for f in range(4):
                            nc.tensor.matmul(
                                ps[:],
                                lhsT=otc[f][:, t4 * 128:(t4 + 1) * 128],
                                rhs=WO[:, f * 1024 + c2 * 512: f * 1024 + c2 * 512 + 512],
                                start=(f == 0), stop=(f == 3))
                        yv = evacpool.tile([128, 512], F32, name="yv", tag="yv")
                        nc.vector.tensor_add(yv[:], ps[:], BO[:, c2 * 512:(c2 + 1) * 512])
                        nc.sync.dma_start(
                            y_d[tt * 128:(tt + 1) * 128, c2 * 512:(c2 + 1) * 512], yv[:])

    nc.compile()
    return nc


def get_program():
    if "nc" not in _cache:
        _cache["nc"] = _build_program()
    return _cache["nc"]


def shard_inputs(x, engagement, mask, qkv_w, qkv_b, out_w, out_b):
    """Build the per-core input maps (host-side layout prep only)."""
    x = np.asarray(x, dtype=np.float32)
    engagement = np.asarray(engagement, dtype=np.float32)
    maskf = np.asarray(mask).astype(np.float32)
    qkv_w = np.asarray(qkv_w, dtype=np.float32)
    qkv_b = np.asarray(qkv_b, dtype=np.float32)
    out_w = np.asarray(out_w, dtype=np.float32)
    out_b = np.asarray(out_b, dtype=np.float32)

    qkvT = qkv_w.T  # [D, 3D]
    outT = out_w.T  # [D, D]
    in_maps = []
    for cix in range(8):
        b, hg = cix // 2, cix % 2
        qcols = qkvT[:, hg * 512:(hg + 1) * 512]
        kcols = qkvT[:, 1024 + hg * 512: 1024 + (hg + 1) * 512]
        sel = np.concatenate([qcols, kcols], axis=1)  # [1024 din, 1024 feats]
        # [d, p, m, f] -> [m, p, d, f] -> [(m p), (d f)]
        wqk = sel.reshape(NDT, 128, 8, 128).transpose(2, 1, 0, 3).reshape(1024, 1024)
        bq = qkv_b[hg * 512:(hg + 1) * 512].reshape(4, 128).T
        bk = qkv_b[1024 + hg * 512: 1024 + (hg + 1) * 512].reshape(4, 128).T
        bo = np.broadcast_to(out_b, (128, 1024)) if hg == 0 else np.zeros((128, 1024), np.float32)
        in_maps.append({
            "xt": np.ascontiguousarray(x[b].T),
            "wqk": np.ascontiguousarray(wqk),
            "wv": np.ascontiguousarray(qkvT[:, 2048 + hg * 512: 2048 + (hg + 1) * 512]),
            "bqk": np.ascontiguousarray(np.concatenate([bq, bk], axis=1)),
            "bv": np.ascontiguousarray(
                np.broadcast_to(qkv_b[2048 + hg * 512: 2048 + (hg + 1) * 512], (128, 512))),
            "eng": np.ascontiguousarray(engagement[b].reshape(NKT, 128).T),
            "maskf": np.ascontiguousarray(maskf[b].reshape(NKT, 128).T),
            "wo": np.ascontiguousarray(outT[hg * 512:(hg + 1) * 512, :]),
            "bo": np.ascontiguousarray(bo),
        })
    return in_maps


def kernel(x, engagement, mask, qkv_w, qkv_b, out_w, out_b):
    global last_results
    nc = get_program()
    in_maps = shard_inputs(x, engagement, mask, qkv_w, qkv_b, out_w, out_b)
    res = run_bass_kernel_spmd(nc, in_maps, list(range(8)))
    last_results = res
    out = np.empty((B, T, D), dtype=np.float32)
    for b in range(B):
        out[b] = res.results[2 * b]["y"] + res.results[2 * b + 1]["y"]
    return out


# revision 6
# speedup vs baseline: 2.8252x; 2.8252x over previous
"""EngagementBiasedMHA on 8 Trainium2 NeuronCores.

Sharding: 4 batches x 2 head-groups (8 heads each).  Each core computes, for
its (batch, head-group):
  - Q^T/K^T projections in [feat, token] layout (feature dim on partitions)
  - V projection in [token, feat] layout, stored per key-tile as
    [V_h | ones(64)] so the PV matmul also produces the softmax denominator
    on 64 partitions
  - attention in transposed layout: S^T = K @ Q^T with keys on partitions, so
    the per-key engagement bias/mask folds into the Exp activation as a
    per-partition bias, and exp(S^T) is already the correct (lhs-contraction)
    layout for the PV matmul
  - O^T = Vhat^T @ P^T accumulated over key tiles (rows 0:64 = head output,
    rows 64:128 = softmax denominator replicated)
  - row-parallel partial output projection y_partial = O_hg @ out_w.T[hg]
Matmul operands are bf16 (4x PE throughput vs fp32); accumulation stays fp32.
Host side: transpose/slice inputs per core, then sum the two partial outputs
per batch (row-parallel unshard).
"""

import sys

if "/opt/trn_rl_repo" not in sys.path:
    sys.path.insert(0, "/opt/trn_rl_repo")

import numpy as np
from concourse import bacc, tile
import concourse.mybir as mybir
from concourse.bass_utils import run_bass_kernel_spmd

F32 = mybir.dt.float32
BF16 = mybir.dt.bfloat16
NP_BF16 = mybir.dt.np(BF16)
AF = mybir.ActivationFunctionType

B, T, D, H = 4, 2048, 1024, 16
HD = 64
HG = 8           # heads per core
NKT = T // 128   # 16 key/token tiles
NQC = T // 512   # 4 query chunks
NDT = D // 128   # 8 d_in tiles
VROW = HG * 128  # 1024 Vhat columns per key tile: per head [V(64) | ones(64)]

_cache = {}

# Results of the most recent run (for the test harness to read exec times).
last_results = None


def _build_program():
    nc = bacc.Bacc("TRN2", target_bir_lowering=False, debug=False, num_devices=8)
    xt_d = nc.declare_dram_parameter("xt", [D, T], BF16, isOutput=False)
    # wqk: row block m*128+p holds, at col d*128+f, weight qkv_w.T[d*128+p, feat(m)+f]
    wqk_d = nc.declare_dram_parameter("wqk", [1024, 1024], BF16, isOutput=False)
    wv_d = nc.declare_dram_parameter("wv", [D, 512], BF16, isOutput=False)
    bqk_d = nc.declare_dram_parameter("bqk", [128, 8], F32, isOutput=False)
    bv_d = nc.declare_dram_parameter("bv", [128, 512], F32, isOutput=False)
    eng_d = nc.declare_dram_parameter("eng", [128, NKT], F32, isOutput=False)
    maskf_d = nc.declare_dram_parameter("maskf", [128, NKT], F32, isOutput=False)
    wo_d = nc.declare_dram_parameter("wo", [512, 1024], BF16, isOutput=False)
    bo_d = nc.declare_dram_parameter("bo", [128, 1024], F32, isOutput=False)
    y_d = nc.declare_dram_parameter("y", [T, D], F32, isOutput=True)

    with tile.TileContext(nc) as tc:
        with (
            tc.tile_pool(name="persist", bufs=1) as persist,
            tc.tile_pool(name="xtpool", bufs=2) as xtpool,
            tc.tile_pool(name="wqkpool", bufs=2) as wqkpool,
            tc.tile_pool(name="wpool", bufs=1) as wpool,
            tc.tile_pool(name="small", bufs=1) as small,
            tc.tile_pool(name="ptpool", bufs=4) as ptpool,
            tc.tile_pool(name="otpool", bufs=6) as otpool,
            tc.tile_pool(name="evacpool", bufs=3) as evacpool,
            tc.tile_pool(name="recpool", bufs=3) as recpool,
            tc.tile_pool(name="psA", bufs=2, space="PSUM") as psA,
            tc.tile_pool(name="psST", bufs=2, space="PSUM") as psST,
            tc.tile_pool(name="psOT", bufs=2, space="PSUM") as psOT,
        ):
            # ---- small inputs ----
            BQK = small.tile([128, 8], F32, name="BQK")
            nc.sync.dma_start(BQK[:], bqk_d[:])
            BV = small.tile([128, 512], F32, name="BV")
            nc.sync.dma_start(BV[:], bv_d[:])
            ENG = small.tile([128, NKT], F32, name="ENG")
            nc.sync.dma_start(ENG[:], eng_d[:])
            MSK = small.tile([128, NKT], F32, name="MSK")
            nc.sync.dma_start(MSK[:], maskf_d[:])
            BO = small.tile([128, 1024], F32, name="BO")
            nc.sync.dma_start(BO[:], bo_d[:])

            # ---- per-key bias: BK = ln(max(eng, 1e-6)) - 1e9 * mask ----
            BK = small.tile([128, NKT], F32, name="BK")
            nc.vector.tensor_scalar_max(BK[:], ENG[:], 1e-6)
            nc.scalar.activation(BK[:], BK[:], AF.Ln)
            MK9 = small.tile([128, NKT], F32, name="MK9")
            nc.vector.tensor_scalar_mul(MK9[:], MSK[:], -1e9)
            nc.vector.tensor_add(BK[:], BK[:], MK9[:])

            # ---- phase 1: projections, chunked over 512-token chunks ----
            QTKT = persist.tile([128, 8 * T], BF16, name="QTKT")
            VHAT = persist.tile([128, NKT * VROW], BF16, name="VHAT")
            nc.vector.memset(VHAT[:], 1.0)
            WV = wpool.tile([128, NDT * 512], BF16, name="WV", tag="wv_wo")
            for d in range(NDT):
                nc.sync.dma_start(WV[:, d * 512:(d + 1) * 512], wv_d[d * 128:(d + 1) * 128, :])

            for c in range(NQC):
                XTc = xtpool.tile([128, NDT * 512], BF16, name="XTc", tag="xtc")
                for d in range(NDT):
                    nc.sync.dma_start(XTc[:, d * 512:(d + 1) * 512],
                                      xt_d[d * 128:(d + 1) * 128, c * 512:(c + 1) * 512])
                # Q^T / K^T features (8 tiles of 128 feats each)
                for m in range(8):
                    WQKm = wqkpool.tile([128, 1024], BF16, name="WQKm", tag="wqk")
                    nc.sync.dma_start(WQKm[:], wqk_d[m * 128:(m + 1) * 128, :])
                    ps = psA.tile([128, 512], F32, name="ps_qk", tag="a")
                    for d in range(NDT):
                        nc.tensor.matmul(
                            ps[:],
                            lhsT=WQKm[:, d * 128:(d + 1) * 128],
                            rhs=XTc[:, d * 512:(d + 1) * 512],
                            start=(d == 0), stop=(d == NDT - 1),
                        )
                    nc.vector.tensor_scalar_add(
                        QTKT[:, m * T + c * 512: m * T + c * 512 + 512],
                        ps[:], BQK[:, m:m + 1])
                # V for this chunk's 4 token tiles
                for t4 in range(4):
                    t = c * 4 + t4
                    ps = psA.tile([128, 512], F32, name="ps_v", tag="a")
                    for d in range(NDT):
                        nc.tensor.matmul(
                            ps[:],
                            lhsT=XTc[:, d * 512 + t4 * 128: d * 512 + (t4 + 1) * 128],
                            rhs=WV[:, d * 512:(d + 1) * 512],
                            start=(d == 0), stop=(d == NDT - 1),
                        )
                    for h in range(HG):
                        nc.vector.tensor_add(
                            VHAT[:, t * VROW + h * 128: t * VROW + h * 128 + 64],
                            ps[:, h * 64:(h + 1) * 64], BV[:, h * 64:(h + 1) * 64])

            WO = wpool.tile([128, 4 * 1024], BF16, name="WO", tag="wv_wo")
            for f in range(4):
                nc.sync.dma_start(WO[:, f * 1024:(f + 1) * 1024], wo_d[f * 128:(f + 1) * 128, :])

            # ---- phase 2+3: attention (transposed layout) + output projection ----
            for qc in range(NQC):
                otc = []
                for hp in range(4):
                    qt = hp
                    ktf = 4 + hp
                    op0 = psOT.tile([128, 512], F32, name="op0", tag="ot")
                    op1 = psOT.tile([128, 512], F32, name="op1", tag="ot")
                    ops = (op0, op1)
                    for kt in range(NKT):
                        st = psST.tile([128, 1024], F32, name="st", tag="st")
                        for sub in range(2):
                            lo = sub * 64
                            nc.tensor.matmul(
                                st[:, sub * 512:(sub + 1) * 512],
                                lhsT=QTKT[lo:lo + 64, ktf * T + kt * 128: ktf * T + (kt + 1) * 128],
                                rhs=QTKT[lo:lo + 64, qt * T + qc * 512: qt * T + qc * 512 + 512],
                                start=True, stop=True)
                        pt = ptpool.tile([128, 1024], BF16, name="pt", tag="pt")
                        nc.scalar.activation(
                            pt[:], st[:], AF.Exp,
                            bias=BK[:, kt:kt + 1], scale=0.125)
                        for sub in range(2):
                            h = 2 * hp + sub
                            nc.tensor.matmul(
                                ops[sub][:],
                                lhsT=VHAT[:, kt * VROW + h * 128: kt * VROW + (h + 1) * 128],
                                rhs=pt[:, sub * 512:(sub + 1) * 512],
                                start=(kt == 0), stop=(kt == NKT - 1))
                    OTc = otpool.tile([128, 512], BF16, name="OTc", tag="otc")
                    for sub in range(2):
                        rec = recpool.tile([64, 512], F32, name="rec", tag="rec")
                        nc.vector.reciprocal(rec[:], ops[sub][64:128, :])
                        nc.vector.tensor_mul(
                            OTc[sub * 64:sub * 64 + 64, :],
                            ops[sub][0:64, :], rec[:])
                    otc.append(OTc)
                # output projection for this 512-token chunk
                for t4 in range(4):
                    tt = qc * 4 + t4
                    for c2 in range(2):
                        ps = psA.tile([128, 512], F32, name="ps_y", tag="a")
                        for f in range(4):
                            nc.tensor.matmul(
                                ps[:],
                                lhsT=otc[f][:, t4 * 128:(t4 + 1) * 128],
                                rhs=WO[:, f * 1024 + c2 * 512: f * 1024 + c2 * 512 + 512],
                                start=(f == 0), stop=(f == 3))
                        yv = evacpool.tile([128, 512], F32, name="yv", tag="yv")
                        nc.vector.tensor_add(yv[:], ps[:], BO[:, c2 * 512:(c2 + 1) * 512])
                        nc.sync.dma_start(
                            y_d[tt * 128:(tt + 1) * 128, c2 * 512:(c2 + 1) * 512], yv[:])

    nc.compile()
    return nc


def get_program():
    if "nc" not in _cache:
        _cache["nc"] = _build_program()
    return _cache["nc"]


def shard_inputs(x, engagement, mask, qkv_w, qkv_b, out_w, out_b):
    """Build the per-core input maps (host-side layout prep only)."""
    x = np.asarray(x, dtype=np.float32)
    engagement = np.asarray(engagement, dtype=np.float32)
    maskf = np.asarray(mask).astype(np.float32)
    qkv_w = np.asarray(qkv_w, dtype=np.float32)
    qkv_b = np.asarray(qkv_b, dtype=np.float32)
    out_w = np.asarray(out_w, dtype=np.float32)
    out_b = np.asarray(out_b, dtype=np.float32)

    qkvT = qkv_w.T  # [D, 3D]
    outT = out_w.T  # [D, D]
    in_maps = []
    for cix in range(8):
        b, hg = cix // 2, cix % 2
        qcols = qkvT[:, hg * 512:(hg + 1) * 512]
        kcols = qkvT[:, 1024 + hg * 512: 1024 + (hg + 1) * 512]
        sel = np.concatenate([qcols, kcols], axis=1)  # [1024 din, 1024 feats]
        # [d, p, m, f] -> [m, p, d, f] -> [(m p), (d f)]
        wqk = sel.reshape(NDT, 128, 8, 128).transpose(2, 1, 0, 3).reshape(1024, 1024)
        bq = qkv_b[hg * 512:(hg + 1) * 512].reshape(4, 128).T
        bk = qkv_b[1024 + hg * 512: 1024 + (hg + 1) * 512].reshape(4, 128).T
        bo = np.broadcast_to(out_b, (128, 1024)) if hg == 0 else np.zeros((128, 1024), np.float32)
        in_maps.append({
            "xt": np.ascontiguousarray(x[b].T).astype(NP_BF16),
            "wqk": np.ascontiguousarray(wqk).astype(NP_BF16),
            "wv": np.ascontiguousarray(
                qkvT[:, 2048 + hg * 512: 2048 + (hg + 1) * 512]).astype(NP_BF16),
            "bqk": np.ascontiguousarray(np.concatenate([bq, bk], axis=1)),
            "bv": np.ascontiguousarray(
                np.broadcast_to(qkv_b[2048 + hg * 512: 2048 + (hg + 1) * 512], (128, 512))),
            "eng": np.ascontiguousarray(engagement[b].reshape(NKT, 128).T),
            "maskf": np.ascontiguousarray(maskf[b].reshape(NKT, 128).T),
            "wo": np.ascontiguousarray(outT[hg * 512:(hg + 1) * 512, :]).astype(NP_BF16),
            "bo": np.ascontiguousarray(bo),
        })
    return in_maps


def kernel(x, engagement, mask, qkv_w, qkv_b, out_w, out_b):
    global last_results
    nc = get_program()
    in_maps = shard_inputs(x, engagement, mask, qkv_w, qkv_b, out_w, out_b)
    res = run_bass_kernel_spmd(nc, in_maps, list(range(8)))
    last_results = res
    out = np.empty((B, T, D), dtype=np.float32)
    for b in range(B):
        out[b] = res.results[2 * b]["y"] + res.results[2 * b + 1]["y"]
    return out


# revision 7
# speedup vs baseline: 3.5334x; 1.2507x over previous
"""EngagementBiasedMHA on 8 Trainium2 NeuronCores.

Sharding: 4 batches x 2 head-groups (8 heads each).  Each core computes, for
its (batch, head-group):
  - Q^T/K^T projections in [feat, token] layout (feature dim on partitions)
  - V projection in [token, feat] layout, stored per key-tile as
    [V_h | ones(64)] so the PV matmul also produces the softmax denominator
    on 64 partitions
  - attention in transposed layout: S^T = K @ Q^T with keys on partitions, so
    the per-key engagement bias/mask folds into the Exp activation as a
    per-partition bias, and exp(S^T) is already the correct (lhs-contraction)
    layout for the PV matmul
  - O^T = Vhat^T @ P^T accumulated over key tiles (rows 0:64 = head output,
    rows 64:128 = softmax denominator replicated)
  - row-parallel partial output projection y_partial = O_hg @ out_w.T[hg]
Matmul operands are bf16 (4x PE throughput vs fp32); accumulation stays fp32.
Host side: transpose/slice inputs per core, then sum the two partial outputs
per batch (row-parallel unshard).
"""

import sys

if "/opt/trn_rl_repo" not in sys.path:
    sys.path.insert(0, "/opt/trn_rl_repo")

import numpy as np
from concourse import bacc, tile
import concourse.mybir as mybir
from concourse.bass_utils import run_bass_kernel_spmd

F32 = mybir.dt.float32
BF16 = mybir.dt.bfloat16
NP_BF16 = mybir.dt.np(BF16)
AF = mybir.ActivationFunctionType

B, T, D, H = 4, 2048, 1024, 16
HD = 64
HG = 8           # heads per core
NKT = T // 128   # 16 key/token tiles
NQC = T // 512   # 4 query chunks
NDT = D // 128   # 8 d_in tiles
VROW = HG * 128  # 1024 Vhat columns per key tile: per head [V(64) | ones(64)]

_cache = {}

# Results of the most recent run (for the test harness to read exec times).
last_results = None


def _build_program():
    nc = bacc.Bacc("TRN2", target_bir_lowering=False, debug=False, num_devices=8)
    xt_d = nc.declare_dram_parameter("xt", [D, T], BF16, isOutput=False)
    # wqk: row block m*128+p holds, at col d*128+f, weight qkv_w.T[d*128+p, feat(m)+f]
    wqk_d = nc.declare_dram_parameter("wqk", [1024, 1024], BF16, isOutput=False)
    wv_d = nc.declare_dram_parameter("wv", [D, 512], BF16, isOutput=False)
    bqk_d = nc.declare_dram_parameter("bqk", [128, 8], F32, isOutput=False)
    bv_d = nc.declare_dram_parameter("bv", [128, 512], F32, isOutput=False)
    eng_d = nc.declare_dram_parameter("eng", [128, NKT], F32, isOutput=False)
    maskf_d = nc.declare_dram_parameter("maskf", [128, NKT], F32, isOutput=False)
    wo_d = nc.declare_dram_parameter("wo", [512, 1024], BF16, isOutput=False)
    bo_d = nc.declare_dram_parameter("bo", [128, 1024], F32, isOutput=False)
    y_d = nc.declare_dram_parameter("y", [T, D], F32, isOutput=True)

    with tile.TileContext(nc) as tc:
        with (
            tc.tile_pool(name="persist", bufs=1) as persist,
            tc.tile_pool(name="xtpool", bufs=2) as xtpool,
            tc.tile_pool(name="wqkpool", bufs=2) as wqkpool,
            tc.tile_pool(name="wpool", bufs=1) as wpool,
            tc.tile_pool(name="small", bufs=1) as small,
            tc.tile_pool(name="ptpool", bufs=4) as ptpool,
            tc.tile_pool(name="otpool", bufs=6) as otpool,
            tc.tile_pool(name="evacpool", bufs=3) as evacpool,
            tc.tile_pool(name="recpool", bufs=3) as recpool,
            tc.tile_pool(name="psmix", bufs=4, space="PSUM") as psmix,
            tc.tile_pool(name="psST", bufs=2, space="PSUM") as psST,
        ):
            # ---- small inputs ----
            BQK = small.tile([128, 8], F32, name="BQK")
            nc.sync.dma_start(BQK[:], bqk_d[:])
            BV = small.tile([128, 512], F32, name="BV")
            nc.sync.dma_start(BV[:], bv_d[:])
            ENG = small.tile([128, NKT], F32, name="ENG")
            nc.sync.dma_start(ENG[:], eng_d[:])
            MSK = small.tile([128, NKT], F32, name="MSK")
            nc.sync.dma_start(MSK[:], maskf_d[:])
            BO = small.tile([128, 1024], F32, name="BO")
            nc.sync.dma_start(BO[:], bo_d[:])

            # ---- per-key bias: BK = ln(max(eng, 1e-6)) - 1e9 * mask ----
            BK = small.tile([128, NKT], F32, name="BK")
            nc.vector.tensor_scalar_max(BK[:], ENG[:], 1e-6)
            nc.scalar.activation(BK[:], BK[:], AF.Ln)
            MK9 = small.tile([128, NKT], F32, name="MK9")
            nc.vector.tensor_scalar_mul(MK9[:], MSK[:], -1e9)
            nc.vector.tensor_add(BK[:], BK[:], MK9[:])

            # ---- phase 1: projections, chunked over 512-token chunks ----
            QTKT = persist.tile([128, 8 * T], BF16, name="QTKT")
            VHAT = persist.tile([128, NKT * VROW], BF16, name="VHAT")
            nc.gpsimd.memset(VHAT[:], 1.0)
            WV = wpool.tile([128, NDT * 512], BF16, name="WV", tag="wv_wo")
            for d in range(NDT):
                nc.sync.dma_start(WV[:, d * 512:(d + 1) * 512], wv_d[d * 128:(d + 1) * 128, :])

            for c in range(NQC):
                XTc = xtpool.tile([128, NDT * 512], BF16, name="XTc", tag="xtc")
                for d in range(NDT):
                    nc.sync.dma_start(XTc[:, d * 512:(d + 1) * 512],
                                      xt_d[d * 128:(d + 1) * 128, c * 512:(c + 1) * 512])
                # Q^T / K^T features (8 tiles of 128 feats each)
                for m in range(8):
                    WQKm = wqkpool.tile([128, 1024], BF16, name="WQKm", tag="wqk")
                    nc.sync.dma_start(WQKm[:], wqk_d[m * 128:(m + 1) * 128, :])
                    ps = psmix.tile([128, 512], F32, name="ps_qk", tag="mix")
                    for d in range(NDT):
                        nc.tensor.matmul(
                            ps[:],
                            lhsT=WQKm[:, d * 128:(d + 1) * 128],
                            rhs=XTc[:, d * 512:(d + 1) * 512],
                            start=(d == 0), stop=(d == NDT - 1),
                        )
                    nc.scalar.activation(
                        QTKT[:, m * T + c * 512: m * T + c * 512 + 512],
                        ps[:], AF.Identity, bias=BQK[:, m:m + 1])
                # V for this chunk's 4 token tiles
                for t4 in range(4):
                    t = c * 4 + t4
                    ps = psmix.tile([128, 512], F32, name="ps_v", tag="mix")
                    for d in range(NDT):
                        nc.tensor.matmul(
                            ps[:],
                            lhsT=XTc[:, d * 512 + t4 * 128: d * 512 + (t4 + 1) * 128],
                            rhs=WV[:, d * 512:(d + 1) * 512],
                            start=(d == 0), stop=(d == NDT - 1),
                        )
                    vslice = VHAT[:, t * VROW:(t + 1) * VROW].rearrange(
                        "p (h c) -> p h c", c=128)[:, :, 0:64]
                    nc.vector.tensor_add(
                        vslice,
                        ps[:].rearrange("p (h c) -> p h c", c=64),
                        BV[:].rearrange("p (h c) -> p h c", c=64))

            WO = wpool.tile([128, 4 * 1024], BF16, name="WO", tag="wv_wo")
            for f in range(4):
                nc.sync.dma_start(WO[:, f * 1024:(f + 1) * 1024], wo_d[f * 128:(f + 1) * 128, :])

            # ---- phase 2+3: attention (transposed layout) + output projection ----
            for qc in range(NQC):
                otc = []
                for hp in range(4):
                    qt = hp
                    ktf = 4 + hp
                    op0 = psmix.tile([128, 512], F32, name="op0", tag="mix")
                    op1 = psmix.tile([128, 512], F32, name="op1", tag="mix")
                    ops = (op0, op1)
                    for kt in range(NKT):
                        st = psST.tile([128, 1024], F32, name="st", tag="st")
                        for sub in range(2):
                            lo = sub * 64
                            nc.tensor.matmul(
                                st[:, sub * 512:(sub + 1) * 512],
                                lhsT=QTKT[lo:lo + 64, ktf * T + kt * 128: ktf * T + (kt + 1) * 128],
                                rhs=QTKT[lo:lo + 64, qt * T + qc * 512: qt * T + qc * 512 + 512],
                                start=True, stop=True)
                        pt = ptpool.tile([128, 1024], BF16, name="pt", tag="pt")
                        nc.scalar.activation(
                            pt[:], st[:], AF.Exp,
                            bias=BK[:, kt:kt + 1], scale=0.125)
                        for sub in range(2):
                            h = 2 * hp + sub
                            nc.tensor.matmul(
                                ops[sub][:],
                                lhsT=VHAT[:, kt * VROW + h * 128: kt * VROW + (h + 1) * 128],
                                rhs=pt[:, sub * 512:(sub + 1) * 512],
                                start=(kt == 0), stop=(kt == NKT - 1))
                    OTc = otpool.tile([128, 512], BF16, name="OTc", tag="otc")
                    for sub in range(2):
                        rec = recpool.tile([64, 512], F32, name="rec", tag="rec")
                        nc.vector.reciprocal_approx_fast(rec[:], ops[sub][64:128, :])
                        nc.vector.tensor_mul(
                            OTc[sub * 64:sub * 64 + 64, :],
                            ops[sub][0:64, :], rec[:])
                    otc.append(OTc)
                # output projection for this 512-token chunk
                for t4 in range(4):
                    tt = qc * 4 + t4
                    for c2 in range(2):
                        ps = psmix.tile([128, 512], F32, name="ps_y", tag="mix")
                        for f in range(4):
                            nc.tensor.matmul(
                                ps[:],
                                lhsT=otc[f][:, t4 * 128:(t4 + 1) * 128],
                                rhs=WO[:, f * 1024 + c2 * 512: f * 1024 + c2 * 512 + 512],
                                start=(f == 0), stop=(f == 3))
                        yv = evacpool.tile([128, 512], F32, name="yv", tag="yv")
                        nc.vector.tensor_add(yv[:], ps[:], BO[:, c2 * 512:(c2 + 1) * 512])
                        nc.sync.dma_start(
                            y_d[tt * 128:(tt + 1) * 128, c2 * 512:(c2 + 1) * 512], yv[:])

    nc.compile()
    return nc


def get_program():
    if "nc" not in _cache:
        _cache["nc"] = _build_program()
    return _cache["nc"]


def shard_inputs(x, engagement, mask, qkv_w, qkv_b, out_w, out_b):
    """Build the per-core input maps (host-side layout prep only)."""
    x = np.asarray(x, dtype=np.float32)
    engagement = np.asarray(engagement, dtype=np.float32)
    maskf = np.asarray(mask).astype(np.float32)
    qkv_w = np.asarray(qkv_w, dtype=np.float32)
    qkv_b = np.asarray(qkv_b, dtype=np.float32)
    out_w = np.asarray(out_w, dtype=np.float32)
    out_b = np.asarray(out_b, dtype=np.float32)

    qkvT = qkv_w.T  # [D, 3D]
    outT = out_w.T  # [D, D]
    in_maps = []
    for cix in range(8):
        b, hg = cix // 2, cix % 2
        qcols = qkvT[:, hg * 512:(hg + 1) * 512]
        kcols = qkvT[:, 1024 + hg * 512: 1024 + (hg + 1) * 512]
        sel = np.concatenate([qcols, kcols], axis=1)  # [1024 din, 1024 feats]
        # [d, p, m, f] -> [m, p, d, f] -> [(m p), (d f)]
        wqk = sel.reshape(NDT, 128, 8, 128).transpose(2, 1, 0, 3).reshape(1024, 1024)
        bq = qkv_b[hg * 512:(hg + 1) * 512].reshape(4, 128).T
        bk = qkv_b[1024 + hg * 512: 1024 + (hg + 1) * 512].reshape(4, 128).T
        bo = np.broadcast_to(out_b, (128, 1024)) if hg == 0 else np.zeros((128, 1024), np.float32)
        in_maps.append({
            "xt": np.ascontiguousarray(x[b].T).astype(NP_BF16),
            "wqk": np.ascontiguousarray(wqk).astype(NP_BF16),
            "wv": np.ascontiguousarray(
                qkvT[:, 2048 + hg * 512: 2048 + (hg + 1) * 512]).astype(NP_BF16),
            "bqk": np.ascontiguousarray(np.concatenate([bq, bk], axis=1)),
            "bv": np.ascontiguousarray(
                np.broadcast_to(qkv_b[2048 + hg * 512: 2048 + (hg + 1) * 512], (128, 512))),
            "eng": np.ascontiguousarray(engagement[b].reshape(NKT, 128).T),
            "maskf": np.ascontiguousarray(maskf[b].reshape(NKT, 128).T),
            "wo": np.ascontiguousarray(outT[hg * 512:(hg + 1) * 512, :]).astype(NP_BF16),
            "bo": np.ascontiguousarray(bo),
        })
    return in_maps


def kernel(x, engagement, mask, qkv_w, qkv_b, out_w, out_b):
    global last_results
    nc = get_program()
    in_maps = shard_inputs(x, engagement, mask, qkv_w, qkv_b, out_w, out_b)
    res = run_bass_kernel_spmd(nc, in_maps, list(range(8)))
    last_results = res
    out = np.empty((B, T, D), dtype=np.float32)
    for b in range(B):
        out[b] = res.results[2 * b]["y"] + res.results[2 * b + 1]["y"]
    return out
